# revision 14
# baseline (speedup 1.0000x reference)
"""DeepseekV3-style MoE block on 8 Trainium2 NeuronCores.

Strategy (expert-parallel, host-side dispatch/combine):
  - Router (sigmoid + top-2 + normalize) computed on host in fp32. Tokens
    are gathered per expert on the host (the "all-to-all dispatch") and
    core e runs expert e's SwiGLU FFN over its gathered token batch
    (padded to a common capacity, 64-token granularity).
  - Shared expert: tensor-parallel 2-way over the hidden dim (HS=1024 ->
    two 512 halves) x data-parallel 4-way over tokens. Core e computes the
    ws-half (e // 4) over token slice (e % 4).
  - Combine: host scatter-adds routed outputs (scaled by routing weights)
    and adds shared partials.

Device kernel (identical program on all 8 cores), v2 structure tuned
from trace analysis of v1:
  - SHARED FFN first: its stage-1 is 8 accumulation chains (one per PSUM
    bank) x F=512 moving columns, consuming one (v13|xts) panel segment
    per dc chunk.  Per-segment PE work (8 MM x 512 cols ~ 1.7-2.0us)
    exceeds per-segment DMA arrival (~1.1us), so the PE is never
    DMA-starved after the first segment -> no mid-stream stalls, no HAM
    re-throttle (every v1 stall cost its duration + ~1.5us of half-clock
    ramp).
  - ROUTED FFN second: by then the whole panel is resident in SBUF.
    Stage-1 runs two passes of 4 weight tiles x (F=512 + F=cap-512)
    chains; stage-2 is mt tiles of 128 tokens x 8 F=512 matmuls.
  - Long zero-weight warmup matmul chain keeps the PE busy from program
    start until the first data segment lands (HAM clock gate needs
    ~3.4us of continuous PE activity for the 2.4GHz state; the v1 warmup
    ended 4us before real work and the PE restarted cold).
  - Stage-1 chains retire in (w1-tile, w3-tile) pairs so silu*mul of the
    first gate tile overlaps the tail matmuls of the pass, and stage-2
    starts while later gate tiles are still being formed.
  - Outputs are written in half-tiles (scalar copies cols 0:512, DMA,
    vector copies 512:1024, DMA) so the final write after the last
    matmul is short.

ALL device input data is packed on the host into ONE bf16 "mega panel"
[128, TOTAL] per core, laid out in exact consumption order:
    [ xts[0] | v13[0] ] | [ v13[dc]|xts[dc] ] x7  |  v2  |
    [ w13[dc] | xtr[dc] ] x8  |  w2
and DMA'd as large descriptors on the (in-order) DMA queue in that order.

PRECISION: bf16 on the wire and bf16 matmuls with fp32 PSUM accumulate;
outputs in bf16. rel err ~5e-3 (gate is 2e-2).
"""

import os
import sys
from contextlib import ExitStack

import numpy as np

if "/opt/trn_rl_repo" not in sys.path and not os.path.isdir(
    os.path.join(os.path.dirname(os.path.abspath(__file__)), "concourse")
):
    sys.path.append("/opt/trn_rl_repo")

D = 1024  # model dim
E = 8  # experts
K = 2  # top-k
H = 512  # expert hidden
HS = 1024  # shared hidden
N_CORES = 8
TP_SHARED = 2  # shared expert split over HS
DP_SHARED = N_CORES // TP_SHARED  # shared expert split over tokens
KC = D // 128  # contraction chunks (dc)
TS = 2048 // DP_SHARED  # shared tokens per core (512)

WARM_C, WARM_F = (
    int(v) for v in os.environ.get("MOE_WARMUP", "12,32").split(",")
)

_NC_CACHE = {}
LAST_RUN = None  # BassKernelResults of the most recent kernel() call


def _build_nc(cap):
    """One-core Bass/Tile program: shared FFN half over TS tokens, then
    expert FFN over `cap` tokens, reading one packed input panel.

    DRAM input (host-packed, bf16) xall [128, TOTAL]:
      per dc: sseg[dc] = v13[dc] (1024 cols = ws1h|ws3h) | xts[dc] (TS)
        (dc=0 is packed xts-first: xts[0] | v13[0], so the first matmul's
        operands arrive earliest)
      then v2 (4096 cols, hc-major); then per dc: rseg[dc] = w13[dc]
      (1024) | xtr[dc] (cap); then w2 (4096, hc-major).
    Outputs: ys [TS, D] bf16 (shared partial), yr [cap, D] bf16
    (unscaled routed).
    """
    import concourse.bacc as bacc
    import concourse.mybir as mybir
    import concourse.tile as tile

    f32 = mybir.dt.float32
    bf16 = mybir.dt.bfloat16
    AF = mybir.ActivationFunctionType

    nc = bacc.Bacc("TRN2", target_bir_lowering=False)

    # --- panel column offsets ---
    SSEG = D + TS
    RSEG = D + cap
    OFF_S = 0
    OFF_V2 = OFF_S + KC * SSEG
    OFF_R = OFF_V2 + 4 * D
    OFF_W2 = OFF_R + KC * RSEG
    TOTAL = OFF_W2 + 4 * D

    # routed stage-1 token groups (chains): F1 = 512, F2 = cap - 512
    F1 = min(cap, 512)
    F2 = cap - F1
    NMT = -(-cap // 128)  # routed stage-2 token tiles

    xall = nc.declare_dram_parameter("xall", [128, TOTAL], bf16, isOutput=False)
    ys = nc.declare_dram_parameter("ys", [TS, D], bf16, isOutput=True)
    yr = nc.declare_dram_parameter("yr", [cap, D], bf16, isOutput=True)

    with ExitStack() as ctx:
        tc = ctx.enter_context(tile.TileContext(nc))
        wpool = ctx.enter_context(tc.tile_pool(name="w", bufs=1))
        gpool = ctx.enter_context(tc.tile_pool(name="g", bufs=1))
        spool = ctx.enter_context(tc.tile_pool(name="s", bufs=2))
        ypool = ctx.enter_context(tc.tile_pool(name="y", bufs=3))
        ps = ctx.enter_context(tc.tile_pool(name="ps", bufs=8, space="PSUM"))

        mega = wpool.tile([128, TOTAL], bf16, tag="mega", name="mega")

        # Input DMA descriptors, spread across both hardware DGE queues
        # (SP + Activation) so descriptor issue (~0.65us each, serial
        # per queue) doesn't gate arrival.  Each queue gets its share in
        # consumption order; the first two descriptors (xts[0], v13[0])
        # are the heads of the two queues so the first matmul's operands
        # land as early as possible.
        def seg(eng, a, b):
            eng.dma_start(mega[:, a:b], xall[:, a:b])

        seg(nc.scalar, OFF_S, OFF_S + TS)  # xts[0]
        seg(nc.sync, OFF_S + TS, OFF_S + SSEG)  # v13[0]
        for dc in range(1, KC):
            eng = nc.scalar if dc % 2 else nc.sync
            seg(eng, OFF_S + dc * SSEG, OFF_S + (dc + 1) * SSEG)
        seg(nc.scalar, OFF_V2, OFF_V2 + 4 * D)  # v2
        for dc in range(KC):
            eng = nc.scalar if dc % 2 else nc.sync
            seg(eng, OFF_R + dc * RSEG, OFF_R + (dc + 1) * RSEG)
        seg(nc.sync, OFF_W2, OFF_W2 + 4 * D)  # w2

        # Warm the PE's HAM clock gate during the initial DMA wait.
        # Program entry is ~7.2us into the NEFF (fixed runtime preamble);
        # the first two data segments complete around ~10.2us.  The
        # warmup chain is sized to span exactly that window (coarse
        # F=128 matmuls, then a fine-grained F=32 tail so the overshoot
        # past first-data-arrival is small), keeping the PE continuously
        # busy into the real stream so it enters it at 2.4GHz.
        warm = wpool.tile([128, 128], bf16, tag="warm", name="warm")
        nc.vector.memset(warm[:], 0.0)
        wp = ps.tile([128, 512], f32, tag="ps", name="wp")
        n_mm = WARM_C + WARM_F
        for i in range(n_mm):
            f = 128 if i < WARM_C else 32
            nc.tensor.matmul(
                wp[:, :f],
                warm[:],
                warm[:, :f],
                start=(i == 0),
                stop=(i == n_mm - 1),
            )

        # ---------------- shared FFN ----------------
        # stage 1: 8 chains (one PSUM bank each), hidden tile j: cols
        # j*128 of v13 (j 0-3 = ws1 half, 4-7 = ws3 half).  Emit order
        # per dc pairs (j, j+4) so gate tiles retire in order.
        jorder = [0, 4, 1, 5, 2, 6, 3, 7]
        hs = [
            ps.tile([128, 512], f32, tag="ps", name=f"hs{j}") for j in range(8)
        ]
        for dc in range(KC):
            wcol = OFF_S + dc * SSEG + (TS if dc == 0 else 0)
            xcol = OFF_S + dc * SSEG + (0 if dc == 0 else D)
            for j in jorder:
                nc.tensor.matmul(
                    hs[j][:, :TS],
                    mega[:, wcol + j * 128 : wcol + (j + 1) * 128],
                    mega[:, xcol : xcol + TS],
                    start=(dc == 0),
                    stop=(dc == KC - 1),
                )
        g_s = [
            gpool.tile([128, TS], bf16, tag=f"gs{k}", name=f"gs{k}")
            for k in range(4)
        ]
        for k in range(4):
            st = spool.tile([128, 512], f32, tag="st", name="st")
            nc.scalar.activation(st[:, :TS], hs[k][:, :TS], AF.Silu)
            nc.vector.tensor_mul(g_s[k][:, :TS], st[:, :TS], hs[k + 4][:, :TS])

        # stage 2: per 128-token tile mt: accumulate over hc, nh inner
        # (2 matmuls per gate lhsT tile, alternating PSUM banks).
        for mt in range(TS // 128):
            r0 = mt * 128
            yp = [
                ps.tile([128, 512], f32, tag="ps", name=f"yps{nh}")
                for nh in range(2)
            ]
            for hc in range(4):
                for nh in range(2):
                    nc.tensor.matmul(
                        yp[nh][:, :512],
                        g_s[hc][:, r0 : r0 + 128],
                        mega[:, OFF_V2 + hc * D + nh * 512 : OFF_V2 + hc * D + (nh + 1) * 512],
                        start=(hc == 0),
                        stop=(hc == 3),
                    )
            y_sb = ypool.tile([128, D], bf16, tag="ysb", name="ysb")
            nc.scalar.activation(y_sb[:, 0:512], yp[0][:, :512], AF.Copy)
            nc.sync.dma_start(ys[r0 : r0 + 128, 0:512], y_sb[:, 0:512])
            nc.vector.tensor_copy(y_sb[:, 512:1024], yp[1][:, :512])
            nc.sync.dma_start(ys[r0 : r0 + 128, 512:1024], y_sb[:, 512:1024])

        # ---------------- routed FFN ----------------
        # stage 1: two passes of 4 weight tiles; per (dc, tile): F1 and
        # F2 chains share the loaded weights.  All data is resident in
        # SBUF by now (the shared FFN covered the DMA window).
        g_r = [
            gpool.tile([128, cap], bf16, tag=f"gr{k}", name=f"gr{k}")
            for k in range(4)
        ]

        def routed_s1_pass(tiles):
            c1 = {}
            c2 = {}
            for j in tiles:
                c1[j] = ps.tile([128, 512], f32, tag="ps", name=f"rc1_{j}")
                if F2:
                    c2[j] = ps.tile([128, 512], f32, tag="ps", name=f"rc2_{j}")
            for dc in range(KC):
                wcol = OFF_R + dc * RSEG
                xcol = OFF_R + dc * RSEG + D
                for j in tiles:
                    w_t = mega[:, wcol + j * 128 : wcol + (j + 1) * 128]
                    nc.tensor.matmul(
                        c1[j][:, :F1],
                        w_t,
                        mega[:, xcol : xcol + F1],
                        start=(dc == 0),
                        stop=(dc == KC - 1),
                    )
                    if F2:
                        nc.tensor.matmul(
                            c2[j][:, :F2],
                            w_t,
                            mega[:, xcol + F1 : xcol + cap],
                            start=(dc == 0),
                            stop=(dc == KC - 1),
                        )
            return c1, c2

        def routed_gate(k, c1, c2):
            # g_r[k] = silu(h1) * h3 with h1 = tile k, h3 = tile k+4
            st = spool.tile([128, 512], f32, tag="st", name="st")
            nc.scalar.activation(st[:, :F1], c1[k][:, :F1], AF.Silu)
            nc.vector.tensor_mul(g_r[k][:, :F1], st[:, :F1], c1[k + 4][:, :F1])
            if F2:
                st2 = spool.tile([128, 512], f32, tag="st", name="st2")
                nc.scalar.activation(st2[:, :F2], c2[k][:, :F2], AF.Silu)
                nc.vector.tensor_mul(
                    g_r[k][:, F1:cap], st2[:, :F2], c2[k + 4][:, :F2]
                )

        cA1, cA2 = routed_s1_pass([0, 4, 1, 5])
        routed_gate(0, cA1, cA2)
        routed_gate(1, cA1, cA2)
        cB1, cB2 = routed_s1_pass([2, 6, 3, 7])
        routed_gate(2, cB1, cB2)
        routed_gate(3, cB1, cB2)

        # stage 2: mt tiles of <=128 tokens
        for mt in range(NMT):
            r0 = mt * 128
            w = min(128, cap - r0)
            yp = [
                ps.tile([128, 512], f32, tag="ps", name=f"ypr{nh}")
                for nh in range(2)
            ]
            for hc in range(4):
                for nh in range(2):
                    nc.tensor.matmul(
                        yp[nh][:w, :512],
                        g_r[hc][:, r0 : r0 + w],
                        mega[:, OFF_W2 + hc * D + nh * 512 : OFF_W2 + hc * D + (nh + 1) * 512],
                        start=(hc == 0),
                        stop=(hc == 3),
                    )
            y_sb = ypool.tile([128, D], bf16, tag="ysb", name="ysb")
            if mt < NMT - 1:
                nc.scalar.activation(y_sb[:w, 0:512], yp[0][:w, :512], AF.Copy)
                nc.sync.dma_start(yr[r0 : r0 + w, 0:512], y_sb[:w, 0:512])
                nc.vector.tensor_copy(y_sb[:w, 512:1024], yp[1][:w, :512])
                nc.sync.dma_start(yr[r0 : r0 + w, 512:1024], y_sb[:w, 512:1024])
            else:
                # final output tile: quarter-column chunks so the last
                # DMA after the last matmul is short
                for q in range(2):
                    c0 = q * 256
                    nc.scalar.activation(
                        y_sb[:w, c0 : c0 + 256], yp[0][:w, c0 : c0 + 256], AF.Copy
                    )
                    nc.sync.dma_start(
                        yr[r0 : r0 + w, c0 : c0 + 256], y_sb[:w, c0 : c0 + 256]
                    )
                for q in range(2):
                    c0 = q * 256
                    nc.vector.tensor_copy(
                        y_sb[:w, 512 + c0 : 768 + c0], yp[1][:w, c0 : c0 + 256]
                    )
                    nc.sync.dma_start(
                        yr[r0 : r0 + w, 512 + c0 : 768 + c0],
                        y_sb[:w, 512 + c0 : 768 + c0],
                    )

    nc.compile()
    return nc


def kernel(x, gate_w, w1, w3, w2, ws1, ws3, ws2):
    global LAST_RUN
    import ml_dtypes
    from concourse.bass_utils import run_bass_kernel_spmd

    x = np.asarray(x, dtype=np.float32)
    gate_w = np.asarray(gate_w, dtype=np.float32)
    w1 = np.asarray(w1, dtype=np.float32)
    w3 = np.asarray(w3, dtype=np.float32)
    w2 = np.asarray(w2, dtype=np.float32)
    ws1 = np.asarray(ws1, dtype=np.float32)
    ws3 = np.asarray(ws3, dtype=np.float32)
    ws2 = np.asarray(ws2, dtype=np.float32)

    wire_np = ml_dtypes.bfloat16

    b, s, d = x.shape
    T = b * s
    xt = np.ascontiguousarray(x.reshape(T, d))
    ts = T // DP_SHARED  # shared-expert token slice per DP group

    # ---- Router on host (fp32, matches the jax reference's selection) ----
    logits = xt @ gate_w  # [T, E]
    with np.errstate(over="ignore"):
        scores = 1.0 / (1.0 + np.exp(-logits, dtype=np.float32))
    top2 = np.argpartition(-scores, 1, axis=1)[:, :2]  # top-2 set per token
    rows = np.arange(T)
    sel_scores = scores[rows[:, None], top2]  # [T, 2]
    norm_w = sel_scores / sel_scores.sum(axis=1, keepdims=True)

    tok_ids = []
    tok_w = []
    sel = np.zeros((T, E), dtype=bool)
    wmat = np.zeros((T, E), dtype=np.float32)
    sel[rows[:, None], top2] = True
    wmat[rows[:, None], top2] = norm_w
    for e in range(E):
        ids = np.nonzero(sel[:, e])[0]
        tok_ids.append(ids)
        tok_w.append(wmat[ids, e])

    max_ne = max(len(ids) for ids in tok_ids)
    cap = max(512, -(-max_ne // 8) * 8)

    # ---- Pack the per-core mega panels (see _build_nc layout) ----
    xtT = np.ascontiguousarray(xt.T).astype(wire_np)  # [D, T]
    w13_all = np.concatenate([w1, w3], axis=2).astype(wire_np)  # [E, D, 2H]
    ws13 = np.stack(
        [
            np.concatenate(
                [ws1[:, hf * H : (hf + 1) * H], ws3[:, hf * H : (hf + 1) * H]],
                axis=1,
            )
            for hf in range(TP_SHARED)
        ]
    ).astype(wire_np)  # [2, D, 2H]
    w2_b = w2.astype(wire_np)  # [E, H, D]
    ws2_b = ws2.astype(wire_np)  # [HS, D]

    SSEG = D + ts
    RSEG = D + cap
    OFF_S = 0
    OFF_V2 = OFF_S + KC * SSEG
    OFF_R = OFF_V2 + 4 * D
    OFF_W2 = OFF_R + KC * RSEG
    TOTAL = OFF_W2 + 4 * D

    in_maps = []
    for e in range(E):
        ids = tok_ids[e]
        sl = e % DP_SHARED
        hf = e // DP_SHARED
        panel = np.zeros((128, TOTAL), dtype=wire_np)
        for dc in range(KC):
            c0 = OFF_S + dc * SSEG
            wof = ts if dc == 0 else 0  # dc=0 seg is xts-first
            xof = 0 if dc == 0 else D
            panel[:, c0 + wof : c0 + wof + D] = ws13[
                hf, dc * 128 : (dc + 1) * 128, :
            ]
            panel[:, c0 + xof : c0 + xof + ts] = xtT[
                dc * 128 : (dc + 1) * 128, sl * ts : (sl + 1) * ts
            ]
        for hc in range(4):
            panel[:, OFF_V2 + hc * D : OFF_V2 + (hc + 1) * D] = ws2_b[
                hf * H + hc * 128 : hf * H + (hc + 1) * 128, :
            ]
        for dc in range(KC):
            c0 = OFF_R + dc * RSEG
            panel[:, c0 : c0 + D] = w13_all[e, dc * 128 : (dc + 1) * 128, :]
            panel[:, c0 + D : c0 + D + len(ids)] = xtT[
                dc * 128 : (dc + 1) * 128, ids
            ]
        for hc in range(4):
            panel[:, OFF_W2 + hc * D : OFF_W2 + (hc + 1) * D] = w2_b[
                e, hc * 128 : (hc + 1) * 128, :
            ]
        in_maps.append({"xall": panel})

    key = (cap, WARM_C, WARM_F)
    nc = _NC_CACHE.get(key)
    if nc is None:
        nc = _build_nc(cap)
        _NC_CACHE[key] = nc

    last_err = None
    for _attempt in range(3):
        try:
            LAST_RUN = run_bass_kernel_spmd(nc, in_maps, list(range(N_CORES)))
            break
        except Exception as err:  # transient NRT/device failures: retry
            last_err = err
    else:
        raise last_err
    results = LAST_RUN.results

    # ---- Combine on host ----
    out = np.zeros((T, d), dtype=np.float32)
    for e in range(E):
        ids = tok_ids[e]
        out[ids] += results[e]["yr"][: len(ids)].astype(np.float32) * tok_w[e][
            :, None
        ]
        sl = e % DP_SHARED
        out[sl * ts : (sl + 1) * ts] += results[e]["ys"].astype(np.float32)
    return out.reshape(b, s, d)


# revision 19
# speedup vs baseline: 1.0652x; 1.0652x over previous
"""DeepseekV3-style MoE block on 8 Trainium2 NeuronCores.

Strategy (expert-parallel, host-side dispatch/combine):
  - Router (sigmoid + top-2 + normalize) computed on host in fp32. Tokens
    are gathered per expert on the host (the "all-to-all dispatch") and
    core e runs expert e's SwiGLU FFN over its gathered token batch
    (padded to a common capacity, 64-token granularity).
  - Shared expert: tensor-parallel 2-way over the hidden dim (HS=1024 ->
    two 512 halves) x data-parallel 4-way over tokens. Core e computes the
    ws-half (e // 4) over token slice (e % 4).
  - Combine: host scatter-adds routed outputs (scaled by routing weights)
    and adds shared partials.

Device kernel (identical program on all 8 cores), v2 structure tuned
from trace analysis of v1:
  - SHARED FFN first: its stage-1 is 8 accumulation chains (one per PSUM
    bank) x F=512 moving columns, consuming one (v13|xts) panel segment
    per dc chunk.  Per-segment PE work (8 MM x 512 cols ~ 1.7-2.0us)
    exceeds per-segment DMA arrival (~1.1us), so the PE is never
    DMA-starved after the first segment -> no mid-stream stalls, no HAM
    re-throttle (every v1 stall cost its duration + ~1.5us of half-clock
    ramp).
  - ROUTED FFN second: by then the whole panel is resident in SBUF.
    Stage-1 runs two passes of 4 weight tiles x (F=512 + F=cap-512)
    chains; stage-2 is mt tiles of 128 tokens x 8 F=512 matmuls.
  - Long zero-weight warmup matmul chain keeps the PE busy from program
    start until the first data segment lands (HAM clock gate needs
    ~3.4us of continuous PE activity for the 2.4GHz state; the v1 warmup
    ended 4us before real work and the PE restarted cold).
  - Stage-1 chains retire in (w1-tile, w3-tile) pairs so silu*mul of the
    first gate tile overlaps the tail matmuls of the pass, and stage-2
    starts while later gate tiles are still being formed.
  - Outputs are written in half-tiles (scalar copies cols 0:512, DMA,
    vector copies 512:1024, DMA) so the final write after the last
    matmul is short.

ALL device input data is packed on the host into ONE bf16 "mega panel"
[128, TOTAL] per core, laid out in exact consumption order:
    [ xts[0] | v13[0] ] | [ v13[dc]|xts[dc] ] x7  |  v2  |
    [ w13[dc] | xtr[dc] ] x8  |  w2
and DMA'd as large descriptors on the (in-order) DMA queue in that order.

PRECISION: bf16 on the wire and bf16 matmuls with fp32 PSUM accumulate;
outputs in bf16. rel err ~5e-3 (gate is 2e-2).
"""

import os
import sys
from contextlib import ExitStack

import numpy as np

if "/opt/trn_rl_repo" not in sys.path and not os.path.isdir(
    os.path.join(os.path.dirname(os.path.abspath(__file__)), "concourse")
):
    sys.path.append("/opt/trn_rl_repo")

D = 1024  # model dim
E = 8  # experts
K = 2  # top-k
H = 512  # expert hidden
HS = 1024  # shared hidden
N_CORES = 8
TP_SHARED = 2  # shared expert split over HS
DP_SHARED = N_CORES // TP_SHARED  # shared expert split over tokens
KC = D // 128  # contraction chunks (dc)
TS = 2048 // DP_SHARED  # shared tokens per core (512)

WARM_C, WARM_F = (
    int(v) for v in os.environ.get("MOE_WARMUP", "30,20").split(",")
)
DMA_Q = os.environ.get("MOE_DMA_Q", "single")

_NC_CACHE = {}
LAST_RUN = None  # BassKernelResults of the most recent kernel() call


def _build_nc(cap):
    """One-core Bass/Tile program: shared FFN half over TS tokens, then
    expert FFN over `cap` tokens, reading one packed input panel.

    DRAM input (host-packed, bf16) xall [128, TOTAL]:
      per dc: sseg[dc] = v13[dc] (1024 cols = ws1h|ws3h) | xts[dc] (TS)
        (dc=0 is packed xts-first: xts[0] | v13[0], so the first matmul's
        operands arrive earliest)
      then v2 (4096 cols, hc-major); then per dc: rseg[dc] = w13[dc]
      (1024) | xtr[dc] (cap); then w2 (4096, hc-major).
    Outputs: ys [TS, D] bf16 (shared partial), yr [cap, D] bf16
    (unscaled routed).
    """
    import concourse.bacc as bacc
    import concourse.mybir as mybir
    import concourse.tile as tile

    f32 = mybir.dt.float32
    bf16 = mybir.dt.bfloat16
    AF = mybir.ActivationFunctionType

    nc = bacc.Bacc("TRN2", target_bir_lowering=False)

    # --- panel column offsets ---
    SSEG = D + TS
    RSEG = D + cap
    OFF_S = 0
    OFF_V2 = OFF_S + KC * SSEG
    OFF_R = OFF_V2 + 4 * D
    OFF_W2 = OFF_R + KC * RSEG
    TOTAL = OFF_W2 + 4 * D

    # routed stage-1 token groups (chains): F1 = 512, F2 = cap - 512
    F1 = min(cap, 512)
    F2 = cap - F1
    NMT = -(-cap // 128)  # routed stage-2 token tiles

    xall = nc.declare_dram_parameter("xall", [128, TOTAL], bf16, isOutput=False)
    ys = nc.declare_dram_parameter("ys", [TS, D], bf16, isOutput=True)
    yr = nc.declare_dram_parameter("yr", [cap, D], bf16, isOutput=True)

    with ExitStack() as ctx:
        tc = ctx.enter_context(tile.TileContext(nc))
        wpool = ctx.enter_context(tc.tile_pool(name="w", bufs=1))
        gpool = ctx.enter_context(tc.tile_pool(name="g", bufs=1))
        spool = ctx.enter_context(tc.tile_pool(name="s", bufs=2))
        ypool = ctx.enter_context(tc.tile_pool(name="y", bufs=3))
        ps = ctx.enter_context(tc.tile_pool(name="ps", bufs=8, space="PSUM"))

        mega = wpool.tile([128, TOTAL], bf16, tag="mega", name="mega")

        # Input DMA descriptors, spread across both hardware DGE queues
        # (SP + Activation) so descriptor issue (~0.65us each, serial
        # per queue) doesn't gate arrival.  Each queue gets its share in
        # consumption order; the first two descriptors (xts[0], v13[0])
        # are the heads of the two queues so the first matmul's operands
        # land as early as possible.
        def seg(eng, a, b):
            eng.dma_start(mega[:, a:b], xall[:, a:b])

        dual = DMA_Q == "dual"
        alt = nc.scalar if dual else nc.sync
        seg(alt, OFF_S, OFF_S + TS)  # xts[0]
        seg(nc.sync, OFF_S + TS, OFF_S + SSEG)  # v13[0]
        for dc in range(1, KC):
            eng = alt if dc % 2 else nc.sync
            seg(eng, OFF_S + dc * SSEG, OFF_S + (dc + 1) * SSEG)
        seg(alt, OFF_V2, OFF_V2 + 4 * D)  # v2
        for dc in range(KC):
            eng = alt if dc % 2 else nc.sync
            seg(eng, OFF_R + dc * RSEG, OFF_R + (dc + 1) * RSEG)
        seg(nc.sync, OFF_W2, OFF_W2 + 4 * D)  # w2

        # Warm the PE's HAM clock gate during the initial DMA wait.
        # Program entry is ~7.2us into the NEFF (fixed runtime preamble);
        # the first two data segments complete around ~10.2us.  The
        # warmup chain is sized to span exactly that window (coarse
        # F=128 matmuls, then a fine-grained F=32 tail so the overshoot
        # past first-data-arrival is small), keeping the PE continuously
        # busy into the real stream so it enters it at 2.4GHz.
        warm = wpool.tile([128, 128], bf16, tag="warm", name="warm")
        nc.vector.memset(warm[:], 0.0)
        wp = ps.tile([128, 512], f32, tag="ps", name="wp")
        n_mm = WARM_C + WARM_F
        for i in range(n_mm):
            f = 128 if i < WARM_C else 32
            nc.tensor.matmul(
                wp[:, :f],
                warm[:],
                warm[:, :f],
                start=(i == 0),
                stop=(i == n_mm - 1),
            )

        # ---------------- shared FFN ----------------
        # stage 1: 8 chains (one PSUM bank each), hidden tile j: cols
        # j*128 of v13 (j 0-3 = ws1 half, 4-7 = ws3 half).  Emit order
        # per dc pairs (j, j+4) so gate tiles retire in order.
        jorder = [0, 4, 1, 5, 2, 6, 3, 7]
        hs = [
            ps.tile([128, 512], f32, tag="ps", name=f"hs{j}") for j in range(8)
        ]
        for dc in range(KC):
            wcol = OFF_S + dc * SSEG + (TS if dc == 0 else 0)
            xcol = OFF_S + dc * SSEG + (0 if dc == 0 else D)
            for j in jorder:
                nc.tensor.matmul(
                    hs[j][:, :TS],
                    mega[:, wcol + j * 128 : wcol + (j + 1) * 128],
                    mega[:, xcol : xcol + TS],
                    start=(dc == 0),
                    stop=(dc == KC - 1),
                )
        g_s = [
            gpool.tile([128, TS], bf16, tag=f"gs{k}", name=f"gs{k}")
            for k in range(4)
        ]
        for k in range(4):
            st = spool.tile([128, 512], f32, tag="st", name="st")
            nc.scalar.activation(st[:, :TS], hs[k][:, :TS], AF.Silu)
            nc.vector.tensor_mul(g_s[k][:, :TS], st[:, :TS], hs[k + 4][:, :TS])

        # ---------------- routed FFN ----------------
        # stage 1: two passes of 4 weight tiles; per (dc, tile): F1 and
        # F2 chains share the loaded weights.  All data is resident in
        # SBUF by now (the shared FFN covered the DMA window).
        g_r = [
            gpool.tile([128, cap], bf16, tag=f"gr{k}", name=f"gr{k}")
            for k in range(4)
        ]

        def routed_s1_pass(tiles):
            c1 = {}
            c2 = {}
            for j in tiles:
                c1[j] = ps.tile([128, 512], f32, tag="ps", name=f"rc1_{j}")
                if F2:
                    c2[j] = ps.tile([128, 512], f32, tag="ps", name=f"rc2_{j}")
            for dc in range(KC):
                wcol = OFF_R + dc * RSEG
                xcol = OFF_R + dc * RSEG + D
                for j in tiles:
                    w_t = mega[:, wcol + j * 128 : wcol + (j + 1) * 128]
                    nc.tensor.matmul(
                        c1[j][:, :F1],
                        w_t,
                        mega[:, xcol : xcol + F1],
                        start=(dc == 0),
                        stop=(dc == KC - 1),
                    )
                    if F2:
                        nc.tensor.matmul(
                            c2[j][:, :F2],
                            w_t,
                            mega[:, xcol + F1 : xcol + cap],
                            start=(dc == 0),
                            stop=(dc == KC - 1),
                        )
            return c1, c2

        def routed_gate(k, c1, c2):
            # g_r[k] = silu(h1) * h3 with h1 = tile k, h3 = tile k+4
            st = spool.tile([128, 512], f32, tag="st", name="st")
            nc.scalar.activation(st[:, :F1], c1[k][:, :F1], AF.Silu)
            nc.vector.tensor_mul(g_r[k][:, :F1], st[:, :F1], c1[k + 4][:, :F1])
            if F2:
                st2 = spool.tile([128, 512], f32, tag="st", name="st2")
                nc.scalar.activation(st2[:, :F2], c2[k][:, :F2], AF.Silu)
                nc.vector.tensor_mul(
                    g_r[k][:, F1:cap], st2[:, :F2], c2[k + 4][:, :F2]
                )

        cA1, cA2 = routed_s1_pass([0, 4, 1, 5])
        routed_gate(0, cA1, cA2)
        routed_gate(1, cA1, cA2)
        cB1, cB2 = routed_s1_pass([2, 6, 3, 7])
        routed_gate(2, cB1, cB2)
        routed_gate(3, cB1, cB2)

        # ---------------- shared FFN stage 2 ----------------
        # (after routed stage 1: the v2 weights have long arrived, and
        # running routed stage 1 early keeps maximum slack between the
        # PE stream and the in-order input DMA queue on slow-DMA runs)
        # stage 2: per 128-token tile mt: accumulate over hc, nh inner
        # (2 matmuls per gate lhsT tile, alternating PSUM banks).
        for mt in range(TS // 128):
            r0 = mt * 128
            yp = [
                ps.tile([128, 512], f32, tag="ps", name=f"yps{nh}")
                for nh in range(2)
            ]
            for hc in range(4):
                for nh in range(2):
                    nc.tensor.matmul(
                        yp[nh][:, :512],
                        g_s[hc][:, r0 : r0 + 128],
                        mega[:, OFF_V2 + hc * D + nh * 512 : OFF_V2 + hc * D + (nh + 1) * 512],
                        start=(hc == 0),
                        stop=(hc == 3),
                    )
            y_sb = ypool.tile([128, D], bf16, tag="ysb", name="ysb")
            nc.scalar.activation(y_sb[:, 0:512], yp[0][:, :512], AF.Copy)
            nc.sync.dma_start(ys[r0 : r0 + 128, 0:512], y_sb[:, 0:512])
            nc.vector.tensor_copy(y_sb[:, 512:1024], yp[1][:, :512])
            nc.sync.dma_start(ys[r0 : r0 + 128, 512:1024], y_sb[:, 512:1024])

        # ---------------- routed FFN stage 2 ----------------
        # stage 2: mt tiles of <=128 tokens
        for mt in range(NMT):
            r0 = mt * 128
            w = min(128, cap - r0)
            yp = [
                ps.tile([128, 512], f32, tag="ps", name=f"ypr{nh}")
                for nh in range(2)
            ]
            for hc in range(4):
                for nh in range(2):
                    nc.tensor.matmul(
                        yp[nh][:w, :512],
                        g_r[hc][:, r0 : r0 + w],
                        mega[:, OFF_W2 + hc * D + nh * 512 : OFF_W2 + hc * D + (nh + 1) * 512],
                        start=(hc == 0),
                        stop=(hc == 3),
                    )
            y_sb = ypool.tile([128, D], bf16, tag="ysb", name="ysb")
            if mt < NMT - 1:
                nc.scalar.activation(y_sb[:w, 0:512], yp[0][:w, :512], AF.Copy)
                nc.sync.dma_start(yr[r0 : r0 + w, 0:512], y_sb[:w, 0:512])
                nc.vector.tensor_copy(y_sb[:w, 512:1024], yp[1][:w, :512])
                nc.sync.dma_start(yr[r0 : r0 + w, 512:1024], y_sb[:w, 512:1024])
            else:
                # final output tile (smallest, w = cap-512): both halves
                # copied concurrently on scalar+vector, then ONE DMA —
                # each DMA_DIRECT2D costs ~0.8us of serial issue time on
                # the queue engine, so fewer issues win at the tail
                nc.scalar.activation(y_sb[:w, 0:512], yp[0][:w, :512], AF.Copy)
                nc.vector.tensor_copy(y_sb[:w, 512:1024], yp[1][:w, :512])
                nc.sync.dma_start(yr[r0 : r0 + w, :], y_sb[:w, :])

    nc.compile()
    return nc


def kernel(x, gate_w, w1, w3, w2, ws1, ws3, ws2):
    global LAST_RUN
    import ml_dtypes
    from concourse.bass_utils import run_bass_kernel_spmd

    x = np.asarray(x, dtype=np.float32)
    gate_w = np.asarray(gate_w, dtype=np.float32)
    w1 = np.asarray(w1, dtype=np.float32)
    w3 = np.asarray(w3, dtype=np.float32)
    w2 = np.asarray(w2, dtype=np.float32)
    ws1 = np.asarray(ws1, dtype=np.float32)
    ws3 = np.asarray(ws3, dtype=np.float32)
    ws2 = np.asarray(ws2, dtype=np.float32)

    wire_np = ml_dtypes.bfloat16

    b, s, d = x.shape
    T = b * s
    xt = np.ascontiguousarray(x.reshape(T, d))
    ts = T // DP_SHARED  # shared-expert token slice per DP group

    # ---- Router on host (fp32, matches the jax reference's selection) ----
    logits = xt @ gate_w  # [T, E]
    with np.errstate(over="ignore"):
        scores = 1.0 / (1.0 + np.exp(-logits, dtype=np.float32))
    top2 = np.argpartition(-scores, 1, axis=1)[:, :2]  # top-2 set per token
    rows = np.arange(T)
    sel_scores = scores[rows[:, None], top2]  # [T, 2]
    norm_w = sel_scores / sel_scores.sum(axis=1, keepdims=True)

    tok_ids = []
    tok_w = []
    sel = np.zeros((T, E), dtype=bool)
    wmat = np.zeros((T, E), dtype=np.float32)
    sel[rows[:, None], top2] = True
    wmat[rows[:, None], top2] = norm_w
    for e in range(E):
        ids = np.nonzero(sel[:, e])[0]
        tok_ids.append(ids)
        tok_w.append(wmat[ids, e])

    max_ne = max(len(ids) for ids in tok_ids)
    cap = max(512, -(-max_ne // 8) * 8)

    # ---- Pack the per-core mega panels (see _build_nc layout) ----
    xtT = np.ascontiguousarray(xt.T).astype(wire_np)  # [D, T]
    w13_all = np.concatenate([w1, w3], axis=2).astype(wire_np)  # [E, D, 2H]
    ws13 = np.stack(
        [
            np.concatenate(
                [ws1[:, hf * H : (hf + 1) * H], ws3[:, hf * H : (hf + 1) * H]],
                axis=1,
            )
            for hf in range(TP_SHARED)
        ]
    ).astype(wire_np)  # [2, D, 2H]
    w2_b = w2.astype(wire_np)  # [E, H, D]
    ws2_b = ws2.astype(wire_np)  # [HS, D]

    SSEG = D + ts
    RSEG = D + cap
    OFF_S = 0
    OFF_V2 = OFF_S + KC * SSEG
    OFF_R = OFF_V2 + 4 * D
    OFF_W2 = OFF_R + KC * RSEG
    TOTAL = OFF_W2 + 4 * D

    in_maps = []
    for e in range(E):
        ids = tok_ids[e]
        sl = e % DP_SHARED
        hf = e // DP_SHARED
        panel = np.zeros((128, TOTAL), dtype=wire_np)
        for dc in range(KC):
            c0 = OFF_S + dc * SSEG
            wof = ts if dc == 0 else 0  # dc=0 seg is xts-first
            xof = 0 if dc == 0 else D
            panel[:, c0 + wof : c0 + wof + D] = ws13[
                hf, dc * 128 : (dc + 1) * 128, :
            ]
            panel[:, c0 + xof : c0 + xof + ts] = xtT[
                dc * 128 : (dc + 1) * 128, sl * ts : (sl + 1) * ts
            ]
        for hc in range(4):
            panel[:, OFF_V2 + hc * D : OFF_V2 + (hc + 1) * D] = ws2_b[
                hf * H + hc * 128 : hf * H + (hc + 1) * 128, :
            ]
        for dc in range(KC):
            c0 = OFF_R + dc * RSEG
            panel[:, c0 : c0 + D] = w13_all[e, dc * 128 : (dc + 1) * 128, :]
            panel[:, c0 + D : c0 + D + len(ids)] = xtT[
                dc * 128 : (dc + 1) * 128, ids
            ]
        for hc in range(4):
            panel[:, OFF_W2 + hc * D : OFF_W2 + (hc + 1) * D] = w2_b[
                e, hc * 128 : (hc + 1) * 128, :
            ]
        in_maps.append({"xall": panel})

    key = (cap, WARM_C, WARM_F, DMA_Q)
    nc = _NC_CACHE.get(key)
    if nc is None:
        nc = _build_nc(cap)
        _NC_CACHE[key] = nc

    last_err = None
    for _attempt in range(3):
        try:
            LAST_RUN = run_bass_kernel_spmd(nc, in_maps, list(range(N_CORES)))
            break
        except Exception as err:  # transient NRT/device failures: retry
            last_err = err
    else:
        raise last_err
    results = LAST_RUN.results

    # ---- Combine on host ----
    out = np.zeros((T, d), dtype=np.float32)
    for e in range(E):
        ids = tok_ids[e]
        out[ids] += results[e]["yr"][: len(ids)].astype(np.float32) * tok_w[e][
            :, None
        ]
        sl = e % DP_SHARED
        out[sl * ts : (sl + 1) * ts] += results[e]["ys"].astype(np.float32)
    return out.reshape(b, s, d)
